# revision 9
# baseline (speedup 1.0000x reference)
"""Multi-head attention (B=16, N=1024, C=384, H=6, D=64) on 8 trn2 cores.

Sharding: data-parallel over batch — each core computes 2 full batches.

Per-core layout strategy (all on one NeuronCore, f32 I/O):
  - x is loaded naturally [n, c] and transposed on PE to xT [c, n] (bf16).
  - qkv weights held in bf16; qkT = w.T @ x (bf16 in, f32r out via the
    ScalarE Identity+bias evacuation) with q,k transposed (d on
    partitions) and v natural [n, d] bf16 (bias added on VectorE).
  - S^T[k, q] = k^T.T @ q^T per head (K=64 contraction, f32r). The two
    512-wide q-chunks run concurrently in the PE array via row tiling
    (rows 0-63 / 64-127) into separate single-bank PSUM tiles, using
    DMA-duplicated copies of q^T/k^T in the upper partitions (prefetched
    one head pair ahead).
  - exp split between ScalarE (native Exp, bf16 out) and VectorE
    (Schraudolph bit-trick: round(x*A+B) as int16 bitcast to bf16,
    ~3% rel err, inside the 2e-2 gate), per-(kt, half) static assignment,
    so the softmax is not ScalarE-bound.
  - PV with an augmented [V_h | 1] stationary tensor: row 64 of the output
    is the softmax denominator, at no extra PE cost; PSUM evacuated by
    ScalarE copies into bf16.
  - normalize: bf16 reciprocal of the sum row, K=1 bf16 matmul broadcasts
    it across 64 partitions into PSUM, multiply fused into the attnT
    (bf16) write.
  - proj (bf16): out[n, c2] = attnT.T @ w_proj + b_proj, per n-tile.

All engines run from in-order queues, so cross-stage overlap must be in
the EMISSION order: batch 1's staging (x loads/transposes, qkv) is
emitted between batch 0's attention head pairs, and batch 0's projection
between batch 1's head pairs, so PE always has dense independent work
while the exps drain, and a single unified PSUM work pool (6 banks deep)
keeps rotation stalls short.
"""

import math
import numpy as np
from contextlib import ExitStack, nullcontext

import concourse.bass as bass
import concourse.mybir as mybir
import concourse.tile as tile
from concourse import bacc
from concourse.bass_utils import run_bass_kernel_spmd
from concourse.masks import make_identity

f32 = mybir.dt.float32
f32r = mybir.dt.float32r
bf16 = mybir.dt.bfloat16
i16 = mybir.dt.int16
EXP = mybir.ActivationFunctionType.Exp
COPY = mybir.ActivationFunctionType.Copy
IDENT = mybir.ActivationFunctionType.Identity
MULT = mybir.AluOpType.mult
ADD = mybir.AluOpType.add

B, N, C = 16, 1024, 384
H, D = 6, 64
NCORES = 8
BL = B // NCORES           # batches per core
HP = H // 2                # head pairs
SCALE = D ** -0.5
P = 128
NT = N // P                # 8 n-tiles
CT = C // P                # 3 c-tiles
KT = N // P                # 8 k-tiles in attention
QC = 2                     # 512-wide q chunks
QW = N // QC               # 512

# Schraudolph exp in bf16-space: exp(s*SCALE) ~ bf16_bits(round(s*A + B))
SCH_A = float((1 << 7) / math.log(2.0) * SCALE)
SCH_B = float(127.0 * (1 << 7) - 5.5)
# (kt, half) slots of each head's 16 exp half-tiles that go to VectorE
# (Schraudolph); the rest run on ScalarE (native exp).
DVE_EXP_SLOTS = frozenset({(0, 0), (0, 1), (1, 0), (2, 1), (3, 0), (4, 1),
                           (5, 0), (6, 1), (7, 0), (7, 1)})


def _r(ap, dt=f32r):
    return ap.bitcast(dt)


def build_nc(repeat=1, hwloop=False, stages="full", dve_slots=None,
             au_eng="act"):
    nc = bacc.Bacc("TRN2", target_bir_lowering=False, debug=False)

    x_d = nc.dram_tensor("x", [BL, N, C], f32, kind="ExternalInput").ap()
    wqkv_d = nc.dram_tensor("w_qkv", [C, 3 * C], f32, kind="ExternalInput").ap()
    bqkv_d = nc.dram_tensor("b_qkv", [3 * C], f32, kind="ExternalInput").ap()
    wproj_d = nc.dram_tensor("w_proj", [C, C], f32, kind="ExternalInput").ap()
    bproj_d = nc.dram_tensor("b_proj", [C], f32, kind="ExternalInput").ap()
    out_d = nc.dram_tensor("out", [BL, N, C], f32, kind="ExternalOutput").ap()

    exp_dve = DVE_EXP_SLOTS if dve_slots is None else dve_slots

    with tile.TileContext(nc) as tc, ExitStack() as ctx:
        consts = ctx.enter_context(tc.tile_pool(name="consts", bufs=1))
        big = ctx.enter_context(tc.tile_pool(name="big", bufs=1))
        work4 = ctx.enter_context(tc.tile_pool(name="work4", bufs=4))
        db = ctx.enter_context(tc.tile_pool(name="db", bufs=2))
        ps_att = ctx.enter_context(tc.tile_pool(name="ps_att", bufs=4, space="PSUM"))
        ps_stg = ctx.enter_context(tc.tile_pool(name="ps_stg", bufs=2, space="PSUM"))
        ps_pv = ctx.enter_context(tc.tile_pool(name="ps_pv", bufs=2, space="PSUM"))

        # ---- constants ----
        ident = consts.tile([P, P], f32)
        make_identity(nc, ident)
        ones64 = consts.tile([P, 64], bf16)
        nc.vector.memset(ones64[:], 1.0)

        wqkv_raw = big.tile([P, CT, 3 * C], f32, tag="pt0")
        for kt in range(CT):
            for h2 in range(4):
                nc.sync.dma_start(
                    wqkv_raw[:, kt, h2 * 288:(h2 + 1) * 288],
                    wqkv_d.rearrange("(kt p) m -> p kt m", p=P)[
                        :, kt, h2 * 288:(h2 + 1) * 288],
                )
        wqkv_sb = consts.tile([P, CT, 3 * C], bf16)
        nc.vector.tensor_copy(wqkv_sb[:], wqkv_raw[:])
        wproj_raw = db.tile([P, CT, C], f32, tag="v_sb")
        for kt in range(CT):
            nc.sync.dma_start(
                wproj_raw[:, kt, :],
                wproj_d.rearrange("(kt p) m -> p kt m", p=P)[:, kt, :],
            )
        wproj_sb = consts.tile([P, CT, C], bf16)
        nc.vector.tensor_copy(wproj_sb[:], wproj_raw[:])
        # per-partition bias for the 6 qk c'-tiles
        bqk_sb = consts.tile([P, 6], f32)
        nc.sync.dma_start(bqk_sb[:], bqkv_d[0:768].rearrange("(t p) -> p t", p=P))
        # broadcast biases (vary along free dim)
        bv_sb = consts.tile([P, C], f32)
        nc.sync.dma_start(bv_sb[:], bqkv_d[None, 768:1152].to_broadcast((P, C)))
        bp_sb = consts.tile([P, C], f32)
        nc.sync.dma_start(bp_sb[:], bproj_d[None, :].to_broadcast((P, C)))

        def a_chunk(b, xb, xT, k):
            """Chunk k of x load + transpose (2 of 6 transpose groups)."""
            if k == 0:
                nc.sync.dma_start(
                    xb[:], x_d[b].rearrange("(t p) c -> p t c", p=P))
            for half, ct in [divmod(2 * k, CT), divmod(2 * k + 1, CT)]:
                g = ps_stg.tile([P, QW], f32, tag="wk",
                                name=f"g{b}_{half}_{ct}")
                for j in range(4):
                    nt = half * 4 + j
                    nc.tensor.transpose(
                        g[:, j * P:(j + 1) * P],
                        xb[:, nt, ct * P:(ct + 1) * P],
                        ident[:],
                    )
                nc.scalar.activation(
                    xT[:, ct, half * QW:(half + 1) * QW], g[:], COPY)

        def stage_a(b):
            xb = big.tile([P, NT, C], f32, tag="xb", name=f"xb{b}")
            xT = db.tile([P, CT, N], bf16, tag="xT", name=f"xT{b}")
            for k in range(3):
                a_chunk(b, xb, xT, k)
            return xT

        def qk_chunk(b, xT, qkT, m):
            """One 128-wide c'-tile of q or k (both 512 chunks)."""
            pss = [ps_stg.tile([P, QW], f32, tag="wk", name=f"qkps{b}_{m}_{i}")
                   for i in range(QC)]
            for kt in range(CT):
                for ch in range(QC):
                    nc.tensor.matmul(
                        pss[ch][:],
                        lhsT=wqkv_sb[:, kt, m * P:(m + 1) * P],
                        rhs=xT[:, kt, ch * QW:(ch + 1) * QW],
                        start=(kt == 0), stop=(kt == CT - 1),
                    )
            for ch in range(QC):
                nc.scalar.activation(
                    qkT[:, m, ch * QW:(ch + 1) * QW], pss[ch][:],
                    IDENT, bias=bqk_sb[:, m:m + 1])

        def stage_b_qk(b, xT):
            qkT = db.tile([P, 6, N], f32r, tag="qkT", name=f"qkT{b}")
            for m in (0, 3, 1, 4, 2, 5):
                qk_chunk(b, xT, qkT, m)
            return qkT

        def v_chunk(b, xT, v_sb, k):
            """Two n-tiles of v (+ ones memset on the first chunk)."""
            if k == 0:
                nc.gpsimd.memset(
                    v_sb[:].rearrange(
                        "p t (h e) -> p t h e", e=D + 1)[:, :, :, D:],
                    1.0)
            for nt in (2 * k, 2 * k + 1):
                ps = ps_stg.tile([P, QW], f32, tag="wk", name=f"vps{b}_{nt}")
                for kt in range(CT):
                    nc.tensor.matmul(
                        ps[:, 0:C],
                        lhsT=xT[:, kt, nt * P:(nt + 1) * P],
                        rhs=wqkv_sb[:, kt, 768:1152],
                        start=(kt == 0), stop=(kt == CT - 1),
                    )
                nc.vector.tensor_tensor(
                    v_sb[:, nt].rearrange(
                        "p (h e) -> p h e", e=D + 1)[:, :, 0:D],
                    ps[:, 0:C].rearrange("p (h e) -> p h e", e=D),
                    bv_sb[:].rearrange("p (h e) -> p h e", e=D),
                    ADD,
                )

        def stage_b_v(b, xT):
            v_sb = db.tile([P, NT, H * (D + 1)], bf16, tag="v_sb",
                           name=f"v{b}")
            for k in range(4):
                v_chunk(b, xT, v_sb, k)
            return v_sb

        def emit_dups(qkT, hp, key):
            """DMA-duplicate q^T/k^T rows for row-tiled S^T concurrency."""
            dups = {}
            for head_i, base in ((0, 0), (1, 64)):
                dbase = 64 - base
                qch = slice(QW, N) if head_i == 0 else slice(0, QW)
                qd = db.tile([P, QW], f32r, tag="dupq", bufs=4,
                             name=f"qd{key}_{hp}_{head_i}")
                nc.gpsimd.dma_start(
                    qd[dbase:dbase + 64, :], qkT[base:base + 64, hp, qch])
                kd = db.tile([P, N], f32r, tag="dup", bufs=4,
                             name=f"kd{key}_{hp}_{head_i}")
                nc.gpsimd.dma_start(
                    kd[dbase:dbase + 64, :], qkT[base:base + 64, 3 + hp, :])
                dups[head_i] = (qd, kd)
            return dups

        def do_exp(dst, src, kt, half):
            """PSUM scores -> SBUF softmax numerators (bf16)."""
            if (kt, half) in exp_dve:
                nc.vector.tensor_scalar(
                    dst.bitcast(i16), src, SCH_A, SCH_B, op0=MULT, op1=ADD)
            else:
                nc.scalar.activation(dst, src, EXP, scale=SCALE)

        def stage_c_hp(qkT, v_sb, attnT, hp, dups=None, hook=None):
            """One head pair of attention: S^T, exp, PV(+sums), normalize.
            hook() is called at 5 safe points to emit one filler chunk."""
            do_pv = stages in ("full", "abcpv")
            do_norm = stages == "full"

            def fire():
                if hook is not None:
                    hook()
            pair_aus = {}
            for head_i, base in ((0, 0), (1, 64)):
                head = 2 * hp + head_i
                if dups is None:
                    # serial chunks (startup pair, dup DMA not ready)
                    k_lo = lambda kt: qkT[base:base + 64, 3 + hp,
                                          kt * P:(kt + 1) * P]
                    q_lo = qkT[base:base + 64, hp, 0:QW]
                    k_hi, hipos = k_lo, (base, 0)
                    q_hi = qkT[base:base + 64, hp, QW:N]
                    lopos = (base, 0)
                else:
                    qd, kd = dups[head_i]
                    lopos, hipos = (0, 0), (64, 0)
                    if head_i == 0:
                        k_lo = lambda kt: qkT[0:64, 3 + hp, kt * P:(kt + 1) * P]
                        q_lo = qkT[0:64, hp, 0:QW]
                        k_hi = lambda kt: kd[64:128, kt * P:(kt + 1) * P]
                        q_hi = qd[64:128, :]
                    else:
                        k_lo = lambda kt: kd[0:64, kt * P:(kt + 1) * P]
                        q_lo = qd[0:64, :]
                        k_hi = lambda kt: qkT[64:128, 3 + hp,
                                              kt * P:(kt + 1) * P]
                        q_hi = qkT[64:128, hp, QW:N]

                pt = big.tile([P, KT, N], bf16, tag=f"pt{head_i}")
                for kt in range(KT):
                    st_lo = ps_att.tile([P, QW], f32, tag="st",
                                        name=f"stl{head}_{kt}")
                    st_hi = ps_att.tile([P, QW], f32, tag="st",
                                        name=f"sth{head}_{kt}")
                    nc.tensor.matmul(
                        st_lo[:], lhsT=_r(k_lo(kt)), rhs=_r(q_lo),
                        tile_position=lopos, start=True, stop=True,
                    )
                    nc.tensor.matmul(
                        st_hi[:], lhsT=_r(k_hi(kt)), rhs=_r(q_hi),
                        tile_position=hipos, start=True, stop=True,
                    )
                    do_exp(pt[:, kt, 0:QW], st_lo[:], kt, 0)
                    do_exp(pt[:, kt, QW:N], st_hi[:], kt, 1)
                fire()

                if not do_pv:
                    continue
                # PV with the augmented [V_h | 1] stationary tensor;
                # kt-outer / ch-inner so each stationary tile loads once
                au = work4.tile([65, N], bf16, tag="attnU")
                pos = [ps_pv.tile([65, QW], f32, tag="pv", name=f"pv{head}_{i}")
                       for i in range(QC)]
                for kt in range(KT):
                    for ch in range(QC):
                        nc.tensor.matmul(
                            pos[ch][:],
                            lhsT=v_sb[:, kt,
                                      head * (D + 1):(head + 1) * (D + 1)],
                            rhs=pt[:, kt, ch * QW:(ch + 1) * QW],
                            start=(kt == 0), stop=(kt == KT - 1),
                        )
                for ch in range(QC):
                    if au_eng == "act":
                        nc.scalar.activation(
                            au[:, ch * QW:(ch + 1) * QW], pos[ch][0:65, :],
                            COPY)
                    else:
                        nc.vector.tensor_copy(
                            au[:, ch * QW:(ch + 1) * QW], pos[ch][0:65, :])
                pair_aus[head_i] = au
                fire()

            # normalize: bf16 recip of the sum row, K=1 bf16 matmul
            # broadcasts it across 64 partitions, multiply into attnT
            if not do_norm:
                return
            for head_i, base in ((0, 0), (1, 64)):
                au = pair_aus[head_i]
                with nc.allow_low_precision(
                        reason="bf16 softmax normalization"):
                    nc.vector.reciprocal(au[64:65, :], au[64:65, :])
                    if head_i == 0:
                        dst = attnT[0:64, hp, :]
                    else:
                        an = db.tile([64, N], bf16, tag="attnN")
                        dst = an[:]
                    for ch in range(QC):
                        rb = ps_pv.tile([P, QW], f32, tag="pv",
                                        name=f"rb{head_i}_{ch}")
                        nc.tensor.matmul(
                            rb[0:64, :],
                            lhsT=ones64[64:65, :],
                            rhs=au[64:65, ch * QW:(ch + 1) * QW],
                            tile_position=(64, 0),
                            start=True, stop=True,
                        )
                        nc.vector.tensor_mul(
                            dst[:, ch * QW:(ch + 1) * QW],
                            au[0:64, ch * QW:(ch + 1) * QW],
                            rb[0:64, :],
                        )
                if head_i == 1:
                    nc.gpsimd.dma_start(attnT[64:128, hp, :], an[:])
            fire()

        def d_chunk(attnT, b, nt):
            stage_d(attnT, b, [nt])

        def stage_d(attnT, b, nts):
            for nt in nts:
                ps = ps_stg.tile([P, QW], f32, tag="wk", name=f"dps{b}_{nt}")
                for ct in range(CT):
                    nc.tensor.matmul(
                        ps[:, 0:C],
                        lhsT=attnT[:, ct, nt * P:(nt + 1) * P],
                        rhs=wproj_sb[:, ct, :],
                        start=(ct == 0), stop=(ct == CT - 1),
                    )
                ob = db.tile([P, C], f32, tag="ob", bufs=4)
                nc.vector.tensor_add(ob[:], ps[:, 0:C], bp_sb[:])
                nc.gpsimd.dma_start(
                    out_d[b].rearrange("(t p) c -> p t c", p=P)[:, nt, :],
                    ob[:],
                )

        from collections import deque

        loop_ctx = tc.For_i(0, repeat, 1) if hwloop else nullcontext(None)
        with loop_ctx:
            for rep in range(1 if hwloop else repeat):
                # startup staging for batch 0 (nothing to hide it under)
                xT0 = stage_a(0)
                qkT0 = stage_b_qk(0, xT0)
                v0 = stage_b_v(0, xT0)
                attnT0 = big.tile([P, HP, N], bf16, tag="attnT0")
                attnT1 = big.tile([P, HP, N], bf16, tag="attnT1")
                if stages == "ab":
                    xT1 = stage_a(1)
                    stage_b_qk(1, xT1)
                    stage_b_v(1, xT1)
                    continue
                # batch-1 tiles, written by filler chunks inside C0
                xb1 = big.tile([P, NT, C], f32, tag="xb", name="xb1")
                xT1 = db.tile([P, CT, N], bf16, tag="xT", name="xT1")
                qkT1 = db.tile([P, 6, N], f32r, tag="qkT", name="qkT1")
                v1 = db.tile([P, NT, H * (D + 1)], bf16, tag="v_sb",
                             name="v1")
                dup_box = {}

                fill = deque()
                for k in range(3):
                    fill.append(lambda k=k: a_chunk(1, xb1, xT1, k))
                fill.append(lambda: qk_chunk(1, xT1, qkT1, 0))
                fill.append(lambda: qk_chunk(1, xT1, qkT1, 3))
                fill.append(
                    lambda: dup_box.setdefault(0, emit_dups(qkT1, 0, "b")))
                fill.append(lambda: qk_chunk(1, xT1, qkT1, 1))
                fill.append(lambda: qk_chunk(1, xT1, qkT1, 4))
                fill.append(
                    lambda: dup_box.setdefault(1, emit_dups(qkT1, 1, "b")))
                fill.append(lambda: qk_chunk(1, xT1, qkT1, 2))
                fill.append(lambda: qk_chunk(1, xT1, qkT1, 5))
                fill.append(
                    lambda: dup_box.setdefault(2, emit_dups(qkT1, 2, "b")))
                for k in range(4):
                    fill.append(lambda k=k: v_chunk(1, xT1, v1, k))

                def hook0():
                    if fill:
                        fill.popleft()()

                # batch 0 attention with batch 1 staging interleaved so the
                # in-order PE queue always has dense, ready filler work
                d01 = emit_dups(qkT0, 1, "a")
                d02 = emit_dups(qkT0, 2, "a")
                stage_c_hp(qkT0, v0, attnT0, 0, dups=None, hook=hook0)
                stage_c_hp(qkT0, v0, attnT0, 1, dups=d01, hook=hook0)
                stage_c_hp(qkT0, v0, attnT0, 2, dups=d02, hook=hook0)
                while fill:
                    fill.popleft()()

                # batch 1 attention with batch 0 projection interleaved
                fill1 = deque()
                if stages == "full":
                    for nt in range(NT):
                        fill1.append(
                            lambda nt=nt: d_chunk(attnT0, 0, nt))

                def hook1():
                    if fill1:
                        fill1.popleft()()

                stage_c_hp(qkT1, v1, attnT1, 0, dups=dup_box.get(0),
                           hook=hook1)
                stage_c_hp(qkT1, v1, attnT1, 1, dups=dup_box.get(1),
                           hook=hook1)
                stage_c_hp(qkT1, v1, attnT1, 2, dups=dup_box.get(2),
                           hook=hook1)
                while fill1:
                    fill1.popleft()()
                if stages == "full":
                    stage_d(attnT1, 1, range(NT))

    nc.compile()
    return nc


_NC_CACHE = {}


def _get_nc():
    if "nc" not in _NC_CACHE:
        _NC_CACHE["nc"] = build_nc()
    return _NC_CACHE["nc"]


def kernel(x, w_qkv, b_qkv, w_proj, b_proj):
    x = np.asarray(x, dtype=np.float32)
    w_qkv = np.asarray(w_qkv, dtype=np.float32)
    b_qkv = np.asarray(b_qkv, dtype=np.float32)
    w_proj = np.asarray(w_proj, dtype=np.float32)
    b_proj = np.asarray(b_proj, dtype=np.float32)

    nc = _get_nc()
    in_maps = [
        {
            "x": np.ascontiguousarray(x[i * BL:(i + 1) * BL]),
            "w_qkv": w_qkv,
            "b_qkv": b_qkv,
            "w_proj": w_proj,
            "b_proj": b_proj,
        }
        for i in range(NCORES)
    ]
    res = run_bass_kernel_spmd(nc, in_maps, list(range(NCORES)))
    return np.concatenate([res.results[i]["out"] for i in range(NCORES)], axis=0)


# revision 10
# speedup vs baseline: 1.0301x; 1.0301x over previous
"""Multi-head attention (B=16, N=1024, C=384, H=6, D=64) on 8 trn2 cores.

Sharding: data-parallel over batch — each core computes 2 full batches.

Per-core layout strategy (all on one NeuronCore, f32 I/O):
  - x is loaded naturally [n, c] and transposed on PE to xT [c, n] (bf16).
  - qkv weights held in bf16; qkT = w.T @ x (bf16 in, f32r out via the
    ScalarE Identity+bias evacuation) with q,k transposed (d on
    partitions) and v natural [n, d] bf16 (bias added on VectorE).
  - S^T[k, q] = k^T.T @ q^T per head (K=64 contraction, f32r). The two
    512-wide q-chunks run concurrently in the PE array via row tiling
    (rows 0-63 / 64-127) into separate single-bank PSUM tiles, using
    DMA-duplicated copies of q^T/k^T in the upper partitions (prefetched
    one head pair ahead).
  - exp split between ScalarE (native Exp, bf16 out) and VectorE
    (Schraudolph bit-trick: round(x*A+B) as int16 bitcast to bf16,
    ~3% rel err, inside the 2e-2 gate), per-(kt, half) static assignment,
    so the softmax is not ScalarE-bound.
  - PV with an augmented [V_h | 1] stationary tensor: row 64 of the output
    is the softmax denominator, at no extra PE cost; PSUM evacuated by
    ScalarE copies into bf16.
  - normalize: bf16 reciprocal of the sum row, K=1 bf16 matmul broadcasts
    it across 64 partitions into PSUM, multiply fused into the attnT
    (bf16) write.
  - proj (bf16): out[n, c2] = attnT.T @ w_proj + b_proj, per n-tile.

All engines run from in-order queues, so cross-stage overlap must be in
the EMISSION order: batch 1's staging (x loads/transposes, qkv) is
emitted between batch 0's attention head pairs, and batch 0's projection
between batch 1's head pairs, so PE always has dense independent work
while the exps drain, and a single unified PSUM work pool (6 banks deep)
keeps rotation stalls short.
"""

import math
import numpy as np
from contextlib import ExitStack, nullcontext

import concourse.bass as bass
import concourse.mybir as mybir
import concourse.tile as tile
from concourse import bacc
from concourse.bass_utils import run_bass_kernel_spmd
from concourse.masks import make_identity

f32 = mybir.dt.float32
f32r = mybir.dt.float32r
bf16 = mybir.dt.bfloat16
i16 = mybir.dt.int16
EXP = mybir.ActivationFunctionType.Exp
COPY = mybir.ActivationFunctionType.Copy
IDENT = mybir.ActivationFunctionType.Identity
MULT = mybir.AluOpType.mult
ADD = mybir.AluOpType.add

B, N, C = 16, 1024, 384
H, D = 6, 64
NCORES = 8
BL = B // NCORES           # batches per core
HP = H // 2                # head pairs
SCALE = D ** -0.5
P = 128
NT = N // P                # 8 n-tiles
CT = C // P                # 3 c-tiles
KT = N // P                # 8 k-tiles in attention
QC = 2                     # 512-wide q chunks
QW = N // QC               # 512

# Schraudolph exp in bf16-space: exp(s*SCALE) ~ bf16_bits(round(s*A + B))
SCH_A = float((1 << 7) / math.log(2.0) * SCALE)
SCH_B = float(127.0 * (1 << 7) - 5.5)
# (kt, half) slots of each head's 16 exp half-tiles that go to VectorE
# (Schraudolph); the rest run on ScalarE (native exp).
DVE_EXP_SLOTS = frozenset({(1, 1), (3, 0), (4, 1), (6, 0), (7, 1)})


def _r(ap, dt=f32r):
    return ap.bitcast(dt)


def build_nc(repeat=1, hwloop=False, stages="full", dve_slots=None):
    nc = bacc.Bacc("TRN2", target_bir_lowering=False, debug=False)

    x_d = nc.dram_tensor("x", [BL, N, C], f32, kind="ExternalInput").ap()
    wqkv_d = nc.dram_tensor("w_qkv", [C, 3 * C], f32, kind="ExternalInput").ap()
    bqkv_d = nc.dram_tensor("b_qkv", [3 * C], f32, kind="ExternalInput").ap()
    wproj_d = nc.dram_tensor("w_proj", [C, C], f32, kind="ExternalInput").ap()
    bproj_d = nc.dram_tensor("b_proj", [C], f32, kind="ExternalInput").ap()
    out_d = nc.dram_tensor("out", [BL, N, C], f32, kind="ExternalOutput").ap()

    exp_dve = DVE_EXP_SLOTS if dve_slots is None else dve_slots

    with tile.TileContext(nc) as tc, ExitStack() as ctx:
        consts = ctx.enter_context(tc.tile_pool(name="consts", bufs=1))
        big = ctx.enter_context(tc.tile_pool(name="big", bufs=1))
        work4 = ctx.enter_context(tc.tile_pool(name="work4", bufs=4))
        db = ctx.enter_context(tc.tile_pool(name="db", bufs=2))
        ps6 = ctx.enter_context(tc.tile_pool(name="ps6", bufs=6, space="PSUM"))
        ps_pv = ctx.enter_context(tc.tile_pool(name="ps_pv", bufs=2, space="PSUM"))

        # ---- constants ----
        ident = consts.tile([P, P], f32)
        make_identity(nc, ident)
        ones64 = consts.tile([P, 64], bf16)
        nc.vector.memset(ones64[:], 1.0)

        wqkv_raw = big.tile([P, CT, 3 * C], f32, tag="pt0")
        for kt in range(CT):
            for h2 in range(4):
                nc.sync.dma_start(
                    wqkv_raw[:, kt, h2 * 288:(h2 + 1) * 288],
                    wqkv_d.rearrange("(kt p) m -> p kt m", p=P)[
                        :, kt, h2 * 288:(h2 + 1) * 288],
                )
        wqkv_sb = consts.tile([P, CT, 3 * C], bf16)
        nc.vector.tensor_copy(wqkv_sb[:], wqkv_raw[:])
        wproj_raw = db.tile([P, CT, C], f32, tag="v_sb")
        for kt in range(CT):
            nc.sync.dma_start(
                wproj_raw[:, kt, :],
                wproj_d.rearrange("(kt p) m -> p kt m", p=P)[:, kt, :],
            )
        wproj_sb = consts.tile([P, CT, C], bf16)
        nc.vector.tensor_copy(wproj_sb[:], wproj_raw[:])
        # per-partition bias for the 6 qk c'-tiles
        bqk_sb = consts.tile([P, 6], f32)
        nc.sync.dma_start(bqk_sb[:], bqkv_d[0:768].rearrange("(t p) -> p t", p=P))
        # broadcast biases (vary along free dim)
        bv_sb = consts.tile([P, C], f32)
        nc.sync.dma_start(bv_sb[:], bqkv_d[None, 768:1152].to_broadcast((P, C)))
        bp_sb = consts.tile([P, C], f32)
        nc.sync.dma_start(bp_sb[:], bproj_d[None, :].to_broadcast((P, C)))

        def stage_a(b):
            """Load x, transpose to xT (bf16)."""
            xb = big.tile([P, NT, C], f32, tag="xb")
            for nt in range(NT):
                nc.sync.dma_start(
                    xb[:, nt, :],
                    x_d[b].rearrange("(t p) c -> p t c", p=P)[:, nt, :],
                )
            xT = db.tile([P, CT, N], bf16, tag="xT")
            for half in range(2):
                for ct in range(CT):
                    g = ps6.tile([P, QW], f32, tag="wk", name=f"g{b}_{half}_{ct}")
                    for j in range(4):
                        nt = half * 4 + j
                        nc.tensor.transpose(
                            g[:, j * P:(j + 1) * P],
                            xb[:, nt, ct * P:(ct + 1) * P],
                            ident[:],
                        )
                    nc.scalar.activation(
                        xT[:, ct, half * QW:(half + 1) * QW], g[:], COPY)
            return xT

        def stage_b_qk(b, xT):
            """qkT = w.T @ x with bias via the ScalarE evacuation."""
            qkT = db.tile([P, 6, N], f32r, tag="qkT")
            # hp0's q,k first so attention can begin while the rest finishes
            for m in (0, 3, 1, 4, 2, 5):
                pss = [ps6.tile([P, QW], f32, tag="wk", name=f"qkps{b}_{m}_{i}")
                       for i in range(QC)]
                for kt in range(CT):
                    for ch in range(QC):
                        nc.tensor.matmul(
                            pss[ch][:],
                            lhsT=wqkv_sb[:, kt, m * P:(m + 1) * P],
                            rhs=xT[:, kt, ch * QW:(ch + 1) * QW],
                            start=(kt == 0), stop=(kt == CT - 1),
                        )
                for ch in range(QC):
                    nc.scalar.activation(
                        qkT[:, m, ch * QW:(ch + 1) * QW], pss[ch][:],
                        IDENT, bias=bqk_sb[:, m:m + 1])
            return qkT

        def stage_b_v(b, xT):
            """v natural [n, (h [d|1])] bf16, ones column per head for the
            PV denominator row."""
            v_sb = db.tile([P, NT, H * (D + 1)], bf16, tag="v_sb")
            nc.gpsimd.memset(
                v_sb[:].rearrange("p t (h e) -> p t h e", e=D + 1)[:, :, :, D:],
                1.0)
            for nt in range(NT):
                ps = ps6.tile([P, QW], f32, tag="wk", name=f"vps{b}_{nt}")
                for kt in range(CT):
                    nc.tensor.matmul(
                        ps[:, 0:C],
                        lhsT=xT[:, kt, nt * P:(nt + 1) * P],
                        rhs=wqkv_sb[:, kt, 768:1152],
                        start=(kt == 0), stop=(kt == CT - 1),
                    )
                nc.vector.tensor_tensor(
                    v_sb[:, nt].rearrange("p (h e) -> p h e", e=D + 1)[:, :, 0:D],
                    ps[:, 0:C].rearrange("p (h e) -> p h e", e=D),
                    bv_sb[:].rearrange("p (h e) -> p h e", e=D),
                    ADD,
                )
            return v_sb

        def emit_dups(qkT, hp, key):
            """DMA-duplicate q^T/k^T rows for row-tiled S^T concurrency."""
            dups = {}
            for head_i, base in ((0, 0), (1, 64)):
                dbase = 64 - base
                qch = slice(QW, N) if head_i == 0 else slice(0, QW)
                qd = db.tile([P, QW], f32r, tag="dupq",
                             name=f"qd{key}_{hp}_{head_i}")
                nc.sync.dma_start(
                    qd[dbase:dbase + 64, :], qkT[base:base + 64, hp, qch])
                kd = db.tile([P, N], f32r, tag="dup",
                             name=f"kd{key}_{hp}_{head_i}")
                nc.sync.dma_start(
                    kd[dbase:dbase + 64, :], qkT[base:base + 64, 3 + hp, :])
                dups[head_i] = (qd, kd)
            return dups

        def do_exp(dst, src, kt, half):
            """PSUM scores -> SBUF softmax numerators (bf16)."""
            if (kt, half) in exp_dve:
                nc.vector.tensor_scalar(
                    dst.bitcast(i16), src, SCH_A, SCH_B, op0=MULT, op1=ADD)
            else:
                nc.scalar.activation(dst, src, EXP, scale=SCALE)

        def stage_c_hp(qkT, v_sb, attnT, hp, dups=None):
            """One head pair of attention: S^T, exp, PV(+sums), normalize."""
            do_pv = stages in ("full", "abcpv")
            do_norm = stages == "full"
            pair_aus = {}
            for head_i, base in ((0, 0), (1, 64)):
                head = 2 * hp + head_i
                if dups is None:
                    # serial chunks (startup pair, dup DMA not ready)
                    k_lo = lambda kt: qkT[base:base + 64, 3 + hp,
                                          kt * P:(kt + 1) * P]
                    q_lo = qkT[base:base + 64, hp, 0:QW]
                    k_hi, hipos = k_lo, (base, 0)
                    q_hi = qkT[base:base + 64, hp, QW:N]
                    lopos = (base, 0)
                else:
                    qd, kd = dups[head_i]
                    lopos, hipos = (0, 0), (64, 0)
                    if head_i == 0:
                        k_lo = lambda kt: qkT[0:64, 3 + hp, kt * P:(kt + 1) * P]
                        q_lo = qkT[0:64, hp, 0:QW]
                        k_hi = lambda kt: kd[64:128, kt * P:(kt + 1) * P]
                        q_hi = qd[64:128, :]
                    else:
                        k_lo = lambda kt: kd[0:64, kt * P:(kt + 1) * P]
                        q_lo = qd[0:64, :]
                        k_hi = lambda kt: qkT[64:128, 3 + hp,
                                              kt * P:(kt + 1) * P]
                        q_hi = qkT[64:128, hp, QW:N]

                pt = big.tile([P, KT, N], bf16, tag=f"pt{head_i}")
                for kt in range(KT):
                    st_lo = ps6.tile([P, QW], f32, tag="wk",
                                     name=f"stl{head}_{kt}")
                    st_hi = ps6.tile([P, QW], f32, tag="wk",
                                     name=f"sth{head}_{kt}")
                    nc.tensor.matmul(
                        st_lo[:], lhsT=_r(k_lo(kt)), rhs=_r(q_lo),
                        tile_position=lopos, start=True, stop=True,
                    )
                    nc.tensor.matmul(
                        st_hi[:], lhsT=_r(k_hi(kt)), rhs=_r(q_hi),
                        tile_position=hipos, start=True, stop=True,
                    )
                    do_exp(pt[:, kt, 0:QW], st_lo[:], kt, 0)
                    do_exp(pt[:, kt, QW:N], st_hi[:], kt, 1)

                if not do_pv:
                    continue
                # PV with the augmented [V_h | 1] stationary tensor;
                # kt-outer / ch-inner so each stationary tile loads once
                au = work4.tile([65, N], bf16, tag="attnU")
                pos = [ps_pv.tile([65, QW], f32, tag="pv", name=f"pv{head}_{i}")
                       for i in range(QC)]
                for kt in range(KT):
                    for ch in range(QC):
                        nc.tensor.matmul(
                            pos[ch][:],
                            lhsT=v_sb[:, kt,
                                      head * (D + 1):(head + 1) * (D + 1)],
                            rhs=pt[:, kt, ch * QW:(ch + 1) * QW],
                            start=(kt == 0), stop=(kt == KT - 1),
                        )
                for ch in range(QC):
                    nc.scalar.activation(
                        au[:, ch * QW:(ch + 1) * QW], pos[ch][0:65, :], COPY)
                pair_aus[head_i] = au

            # normalize: bf16 recip of the sum row, K=1 bf16 matmul
            # broadcasts it across 64 partitions, multiply into attnT
            if not do_norm:
                return
            for head_i, base in ((0, 0), (1, 64)):
                au = pair_aus[head_i]
                with nc.allow_low_precision(
                        reason="bf16 softmax normalization"):
                    nc.vector.reciprocal(au[64:65, :], au[64:65, :])
                    if head_i == 0:
                        dst = attnT[0:64, hp, :]
                    else:
                        an = db.tile([64, N], bf16, tag="attnN")
                        dst = an[:]
                    for ch in range(QC):
                        rb = ps_pv.tile([P, QW], f32, tag="pv",
                                        name=f"rb{head_i}_{ch}")
                        nc.tensor.matmul(
                            rb[0:64, :],
                            lhsT=ones64[64:65, :],
                            rhs=au[64:65, ch * QW:(ch + 1) * QW],
                            tile_position=(64, 0),
                            start=True, stop=True,
                        )
                        nc.vector.tensor_mul(
                            dst[:, ch * QW:(ch + 1) * QW],
                            au[0:64, ch * QW:(ch + 1) * QW],
                            rb[0:64, :],
                        )
                if head_i == 1:
                    nc.sync.dma_start(attnT[64:128, hp, :], an[:])

        def stage_d(attnT, b, nts):
            for nt in nts:
                ps = ps6.tile([P, QW], f32, tag="wk", name=f"dps{b}_{nt}")
                for ct in range(CT):
                    nc.tensor.matmul(
                        ps[:, 0:C],
                        lhsT=attnT[:, ct, nt * P:(nt + 1) * P],
                        rhs=wproj_sb[:, ct, :],
                        start=(ct == 0), stop=(ct == CT - 1),
                    )
                ob = db.tile([P, C], f32, tag="ob", bufs=4)
                nc.vector.tensor_add(ob[:], ps[:, 0:C], bp_sb[:])
                nc.sync.dma_start(
                    out_d[b].rearrange("(t p) c -> p t c", p=P)[:, nt, :],
                    ob[:],
                )

        loop_ctx = tc.For_i(0, repeat, 1) if hwloop else nullcontext(None)
        with loop_ctx:
            for rep in range(1 if hwloop else repeat):
                # startup staging for batch 0 (nothing to hide it under)
                xT0 = stage_a(0)
                qkT0 = stage_b_qk(0, xT0)
                v0 = stage_b_v(0, xT0)
                attnT0 = big.tile([P, HP, N], bf16, tag="attnT0")
                attnT1 = big.tile([P, HP, N], bf16, tag="attnT1")
                if stages == "ab":
                    xT1 = stage_a(1)
                    stage_b_qk(1, xT1)
                    stage_b_v(1, xT1)
                    continue
                # batch 0 attention with batch 1 staging interleaved so the
                # in-order PE queue has dense work while the exps drain
                stage_c_hp(qkT0, v0, attnT0, 0, dups=None)
                d01 = emit_dups(qkT0, 1, "a")
                xT1 = stage_a(1)
                stage_c_hp(qkT0, v0, attnT0, 1, dups=d01)
                d02 = emit_dups(qkT0, 2, "a")
                qkT1 = stage_b_qk(1, xT1)
                stage_c_hp(qkT0, v0, attnT0, 2, dups=d02)
                v1 = stage_b_v(1, xT1)
                d10 = emit_dups(qkT1, 0, "b")
                # batch 1 attention with batch 0 projection interleaved
                stage_c_hp(qkT1, v1, attnT1, 0, dups=d10)
                d11 = emit_dups(qkT1, 1, "b")
                if stages == "full":
                    stage_d(attnT0, 0, range(0, 4))
                stage_c_hp(qkT1, v1, attnT1, 1, dups=d11)
                d12 = emit_dups(qkT1, 2, "b")
                if stages == "full":
                    stage_d(attnT0, 0, range(4, NT))
                stage_c_hp(qkT1, v1, attnT1, 2, dups=d12)
                if stages == "full":
                    stage_d(attnT1, 1, range(NT))

    nc.compile()
    return nc


_NC_CACHE = {}


def _get_nc():
    if "nc" not in _NC_CACHE:
        _NC_CACHE["nc"] = build_nc()
    return _NC_CACHE["nc"]


def kernel(x, w_qkv, b_qkv, w_proj, b_proj):
    x = np.asarray(x, dtype=np.float32)
    w_qkv = np.asarray(w_qkv, dtype=np.float32)
    b_qkv = np.asarray(b_qkv, dtype=np.float32)
    w_proj = np.asarray(w_proj, dtype=np.float32)
    b_proj = np.asarray(b_proj, dtype=np.float32)

    nc = _get_nc()
    in_maps = [
        {
            "x": np.ascontiguousarray(x[i * BL:(i + 1) * BL]),
            "w_qkv": w_qkv,
            "b_qkv": b_qkv,
            "w_proj": w_proj,
            "b_proj": b_proj,
        }
        for i in range(NCORES)
    ]
    res = run_bass_kernel_spmd(nc, in_maps, list(range(NCORES)))
    return np.concatenate([res.results[i]["out"] for i in range(NCORES)], axis=0)
